# revision 24
# baseline (speedup 1.0000x reference)
"""Trainium2 Bass kernel for a 6-layer GRU network (B=256, T=512, I=28, H=128, O=10).

Strategy: data-parallel across 8 NeuronCores (batch 256 -> 32 per core),
with a 6-layer WAVEFRONT schedule inside each core: at wavefront step w,
layer l processes timestep t = w - 8*l.  The six layers are split into two
independent groups of three (layers 0-2 / 3-5) whose dependency chains
interleave on the engines, and all gate elementwise work is batched across
each group's three layers into [128, 96]-wide ops (vs [128, 32] per-layer).

Per group-step:
  - PSUM "rz" tile [128, 2steps x 3layers x 2gates x 32] accumulates
    bias (K=6 selector matmul, start=True) + input projection (chunked,
    strided dest, start=False) + recurrent W_hh matmuls (start=False) so
    ONE sigmoid op reads a contiguous [128,192] tile and emits bf16 SBUF.
  - n-gate: gxn PSUM tile (bias + input proj), ghn PSUM tile (bias +
    recurrent mm); hn2 = ghn * r and nin = hn2 + gxn on GpSimd; tanh on
    ScalarE; h-update (d = h-n, e = z*d, h = n+e) on DVE in bf16 SBUF
    (4x fast mode).
  - h state lives in per-layer SBUF rings [128, L, 16, 32] indexed by
    wavefront slot (w % 16), so the batched 3-layer h-update writes one
    strided AP.
Final FC + log_softmax identical to the data-parallel baseline.
"""

import numpy as np

H = 128
I_DIM = 28
L = 6
O = 10
B = 256
T = 512
NCORES = 8
PB = B // NCORES   # 32 batch rows per core
D_OFF = 8          # wavefront offset between consecutive layers
RING = 16          # h-state ring depth (slots of PB cols per layer)
GRPS = ([0, 1, 2], [3, 4, 5])

_CACHE = {}


def _build(t_steps, dt_mm_name="bfloat16"):
    from contextlib import ExitStack

    import concourse.bass as bass  # noqa: F401
    import concourse.tile as tile
    from concourse import bacc, mybir

    f32 = mybir.dt.float32
    bf16 = mybir.dt.bfloat16
    dt_mm = getattr(mybir.dt, dt_mm_name)
    AF = mybir.ActivationFunctionType
    ALU = mybir.AluOpType

    assert t_steps % 2 == 0
    w_end = t_steps + (L - 1) * D_OFF  # wavefront length

    nc = bacc.Bacc("TRN2", target_bir_lowering=False, debug=False)

    xT = nc.dram_tensor("xT", [I_DIM, PB * t_steps], dt_mm, kind="ExternalInput")
    wih0 = nc.dram_tensor("wih0", [I_DIM, 3 * H], dt_mm, kind="ExternalInput")
    wih = nc.dram_tensor("wih", [H, (L - 1) * 3 * H], dt_mm, kind="ExternalInput")
    whh = nc.dram_tensor("whh", [H, L * 3 * H], dt_mm, kind="ExternalInput")
    # rz bias rows per group: [6, H] (row k = layer grp[k//2], gate k%2 (r/z))
    brz_a = nc.dram_tensor("brz_a", [6, H], dt_mm, kind="ExternalInput")
    brz_b = nc.dram_tensor("brz_b", [6, H], dt_mm, kind="ExternalInput")
    erz = nc.dram_tensor("erz", [6, 2 * 3 * 2 * PB], dt_mm, kind="ExternalInput")
    bihn_a = nc.dram_tensor("bihn_a", [3, H], dt_mm, kind="ExternalInput")
    bihn_b = nc.dram_tensor("bihn_b", [3, H], dt_mm, kind="ExternalInput")
    e3 = nc.dram_tensor("e3", [3, 2 * 3 * PB], dt_mm, kind="ExternalInput")
    bhhn_a = nc.dram_tensor("bhhn_a", [3, H], dt_mm, kind="ExternalInput")
    bhhn_b = nc.dram_tensor("bhhn_b", [3, H], dt_mm, kind="ExternalInput")
    fcw = nc.dram_tensor("fcw", [H, O], dt_mm, kind="ExternalInput")
    fcb = nc.dram_tensor("fcb", [1, O], dt_mm, kind="ExternalInput")
    y = nc.dram_tensor("y", [PB, O], f32, kind="ExternalOutput")

    with tile.TileContext(nc) as tc, ExitStack() as ctx:
        consts = ctx.enter_context(tc.tile_pool(name="consts", bufs=1))
        # One persistent PSUM pool per group: rz ring (2 banks) + gxn ring
        # (1 bank) + ghn ping-pong (0.5 bank) = 4 banks; x2 groups = 8 banks.
        # Ring slots are padded so no matmul dest window crosses a bank.
        ps_pool = [
            ctx.enter_context(tc.tile_pool(name=f"ps_pool{g}", bufs=1, space="PSUM"))
            for g in range(2)
        ]
        rz_t = []
        ngate_t = []
        for g in range(2):
            rz = ps_pool[g].tile([H, 4, 8, PB], f32, tag=f"rz{g}", name=f"rz{g}")
            # combined n-gate ring: ghn at even element-pairs, gxn at odd
            ngate = ps_pool[g].tile([H, 4, 4, PB, 2], f32, tag=f"ng{g}",
                                    name=f"ng{g}")
            rz_t.append(rz)
            ngate_t.append(ngate)
        ew_pool = ctx.enter_context(tc.tile_pool(name="ew", bufs=3))
        scratch = ctx.enter_context(tc.tile_pool(name="scratch", bufs=3))

        # ---- load constants ----
        xT_sb = consts.tile([I_DIM, PB * t_steps], dt_mm, tag="xT_sb")
        nc.gpsimd.dma_start(xT_sb[:], xT.ap())
        wih0_sb = consts.tile([I_DIM, 3 * H], dt_mm, tag="wih0_sb")
        nc.gpsimd.dma_start(wih0_sb[:], wih0.ap())
        wih_sb = consts.tile([H, (L - 1) * 3 * H], dt_mm, tag="wih_sb")
        nc.gpsimd.dma_start(wih_sb[:], wih.ap())
        whh_sb = consts.tile([H, L * 3 * H], dt_mm, tag="whh_sb")
        nc.gpsimd.dma_start(whh_sb[:], whh.ap())
        brz_sb = [consts.tile([6, H], dt_mm, tag=f"brz{g}_sb", name=f"brz{g}_sb") for g in range(2)]
        nc.gpsimd.dma_start(brz_sb[0][:], brz_a.ap())
        nc.gpsimd.dma_start(brz_sb[1][:], brz_b.ap())
        erz_sb = consts.tile([6, 2 * 3 * 2 * PB], dt_mm, tag="erz_sb")
        nc.gpsimd.dma_start(erz_sb[:], erz.ap())
        bihn_sb = [consts.tile([3, H], dt_mm, tag=f"bihn{g}_sb", name=f"bihn{g}_sb") for g in range(2)]
        nc.gpsimd.dma_start(bihn_sb[0][:], bihn_a.ap())
        nc.gpsimd.dma_start(bihn_sb[1][:], bihn_b.ap())
        e3_sb = consts.tile([3, 2 * 3 * PB], dt_mm, tag="e3_sb")
        nc.gpsimd.dma_start(e3_sb[:], e3.ap())
        bhhn_sb = [consts.tile([3, H], dt_mm, tag=f"bhhn{g}_sb", name=f"bhhn{g}_sb") for g in range(2)]
        nc.gpsimd.dma_start(bhhn_sb[0][:], bhhn_a.ap())
        nc.gpsimd.dma_start(bhhn_sb[1][:], bhhn_b.ap())
        fcw_sb = consts.tile([H, O], dt_mm, tag="fcw_sb")
        nc.gpsimd.dma_start(fcw_sb[:], fcw.ap())
        fcb_sb = consts.tile([1, O], dt_mm, tag="fcb_sb")
        nc.gpsimd.dma_start(fcb_sb[:], fcb.ap())

        zeros_sb = consts.tile([H, PB], dt_mm, tag="zeros_sb")
        nc.vector.memset(zeros_sb[:], 0.0)
        ones_sb = consts.tile([1, PB], dt_mm, tag="ones_sb")
        nc.vector.memset(ones_sb[:], 1.0)

        # h-state rings: [128, layer, slot, batch]; slot = w % RING
        h_ring = consts.tile([H, L, RING, PB], dt_mm, tag="h_ring")
        nc.vector.memset(h_ring[:], 0.0)

        # sigmoid outputs interleaved with zeros for the nin scan:
        # [H, gate-major 6 sub-slots, PB, 2]; odd positions get r/z, even
        # positions stay 0 forever (scan reset lanes)
        rsc = [consts.tile([H, 6, PB, 2], dt_mm, tag=f"rsc{g}", name=f"rsc{g}")
               for g in range(2)]
        nc.vector.memset(rsc[0][:], 0.0)
        nc.vector.memset(rsc[1][:], 0.0)

        def whh_g(layer, g):
            return whh_sb[:, (layer * 3 + g) * H:(layer * 3 + g + 1) * H]

        def wih_g(layer, g):
            if layer == 0:
                return wih0_sb[:, g * H:(g + 1) * H]
            base = ((layer - 1) * 3 + g) * H
            return wih_sb[:, base:base + H]

        def emit_refill(g, grp, wf):
            """Fill rz + gxn ring slots for steps wf, wf+1 (bias + inproj)."""
            rf = wf % 4
            nc.tensor.matmul(rz_t[g][:, rf:rf + 2, 0:6, :],
                             brz_sb[g][:], erz_sb[:],
                             start=True, stop=False)
            nc.tensor.matmul(ngate_t[g][:, rf:rf + 2, 0:3, :, 1],
                             bihn_sb[g][:], e3_sb[:],
                             start=True, stop=False)
            for j, l in enumerate(grp):
                if l == 0:
                    tc0 = min(wf, t_steps - 2)
                    rhs = xT_sb[:, tc0 * PB:(tc0 + 2) * PB]
                else:
                    ps = (wf - D_OFF) % RING  # even, no wrap
                    rhs = h_ring[:, l - 1, ps:ps + 2, :]
                for gate in range(2):
                    nc.tensor.matmul(rz_t[g][:, rf:rf + 2, gate * 3 + j, :],
                                     wih_g(l, gate), rhs,
                                     start=False, stop=False,
                                     skip_group_check=True)
                nc.tensor.matmul(ngate_t[g][:, rf:rf + 2, j, :, 1],
                                 wih_g(l, 2), rhs,
                                 start=False, stop=True,
                                 skip_group_check=True)

        for g, grp in enumerate(GRPS):
            emit_refill(g, grp, 0)
            emit_refill(g, grp, 2)

        for w in range(w_end):
            s2 = w % 2
            r4 = w % 4
            slot = w % RING
            for g, grp in enumerate(GRPS):
                # ---- per-step recurrent matmuls ----
                # rz mms first: the sigmoid (critical chain) waits only on them
                ng = ngate_t[g]
                # per-slot ghn bias into even element-pairs
                nc.tensor.matmul(ng[:, r4, 0:3, :, 0], bhhn_sb[g][:],
                                 e3_sb[:, 0:3 * PB], start=True, stop=False)
                prev_slot = (w - 1) % RING
                hprevs = []
                for j, l in enumerate(grp):
                    t_l = w - D_OFF * l
                    if t_l == 0:
                        hprev = zeros_sb[:]
                    else:
                        hprev = h_ring[:, l, prev_slot, :]
                    hprevs.append(hprev)
                    nc.tensor.matmul(rz_t[g][:, r4, j, :], whh_g(l, 0), hprev,
                                     start=False, stop=True,
                                     skip_group_check=True)
                    nc.tensor.matmul(rz_t[g][:, r4, 3 + j, :], whh_g(l, 1), hprev,
                                     start=False, stop=True,
                                     skip_group_check=True)
                for j, l in enumerate(grp):
                    nc.tensor.matmul(ng[:, r4, j, :, 0], whh_g(l, 2), hprevs[j],
                                     start=False, stop=True,
                                     skip_group_check=True)

                # ---- gates ----
                nc.scalar.activation(rsc[g][:, :, :, 1],
                                     rz_t[g][:, r4, 0:6, :], AF.Sigmoid)
                z_view = rsc[g][:, 3:6, :, 1]
                # fused nin = r*ghn + gxn via pairwise scan:
                #   even lane: state = 0*state + ghn   (reset to ghn)
                #   odd lane:  state = r*ghn + gxn     (= nin)
                nin_sc = ew_pool.tile([H, 3, PB, 2], dt_mm, tag=f"nin{g}",
                                      name=f"nin{g}")
                nc.vector.tensor_tensor_scan(
                    nin_sc[:].rearrange("p a b c -> p (a b c)"),
                    rsc[g][:, 0:3, :, :].rearrange("p a b c -> p (a b c)"),
                    ng[:, r4, 0:3, :, :].rearrange("p a b c -> p (a b c)"),
                    0.0, op0=ALU.mult, op1=ALU.add)
                # off-critical-chain pieces of the h update (Pool, all-SBUF):
                #   u  = z * h_prev,  z' = 1 - z
                hprev3 = h_ring[:, grp[0]:grp[0] + 3, prev_slot, :]
                u_sb = ew_pool.tile([H, 3, PB], dt_mm, tag=f"u{g}", name=f"u{g}")
                nc.gpsimd.tensor_tensor(u_sb[:], z_view, hprev3, op=ALU.mult)
                zc_sb = ew_pool.tile([H, 3, PB], dt_mm, tag=f"zc{g}", name=f"zc{g}")
                nc.gpsimd.tensor_scalar(zc_sb[:], z_view, -1.0, 1.0,
                                        op0=ALU.mult, op1=ALU.add)
                n_sb = ew_pool.tile([H, 3, PB], dt_mm, tag=f"n{g}", name=f"n{g}")
                nc.scalar.activation(n_sb[:], nin_sc[:, :, :, 1], AF.Tanh)
                # on-chain tail: h = u + (1-z)*n
                v_sb = ew_pool.tile([H, 3, PB], dt_mm, tag=f"v{g}", name=f"v{g}")
                nc.vector.tensor_tensor(v_sb[:], zc_sb[:], n_sb[:], op=ALU.mult)
                nc.vector.tensor_tensor(h_ring[:, grp[0]:grp[0] + 3, slot, :],
                                        u_sb[:], v_sb[:], op=ALU.add)
                # zero the slot a layer will read as h(-1) at its t=0
                for l in grp:
                    if l > 0 and w == D_OFF * l - 1:
                        nc.vector.memset(h_ring[:, l, slot, :], 0.0)

            # ---- refill ring slots 2 steps AHEAD (the ring is depth 4:
            # slots of steps w-2/w-1, already emitted), AFTER both groups'
            # per-step ops so refills never head-of-line-block a group's
            # recurrent matmuls in the PE queue ----
            if s2 == 0 and w >= 2:
                for g, grp in enumerate(GRPS):
                    emit_refill(g, grp, w + 2)

        # ---- FC + log_softmax on h(L-1, T-1) ----
        h_last = h_ring[:, L - 1, (w_end - 1) % RING, :]
        # reuse an untouched pad region of group 1's ngate bank for the logits
        logits_ps = ngate_t[1][0:PB, 0, 3, 0:5, :].rearrange("p a b -> p (a b)")
        nc.tensor.matmul(logits_ps, h_last, fcw_sb[:], start=True, stop=False)
        nc.tensor.matmul(logits_ps, ones_sb[:], fcb_sb[:],
                         start=False, stop=True, skip_group_check=True)
        mx_t = scratch.tile([PB, 1], f32, tag="mx")
        nc.vector.reduce_max(mx_t[:], logits_ps, axis=mybir.AxisListType.X)
        xm_t = scratch.tile([PB, O], f32, tag="xm")
        nc.vector.tensor_scalar(xm_t[:], logits_ps, mx_t[:], None,
                                op0=ALU.subtract)
        ex_t = scratch.tile([PB, O], f32, tag="ex")
        sum_t = scratch.tile([PB, 1], f32, tag="sum")
        nc.scalar.activation(ex_t[:], xm_t[:], AF.Exp, accum_out=sum_t[:])
        ls_t = scratch.tile([PB, 1], f32, tag="ls")
        nc.scalar.activation(ls_t[:], sum_t[:], AF.Ln)
        out_t = scratch.tile([PB, O], f32, tag="out")
        nc.vector.tensor_scalar(out_t[:], xm_t[:], ls_t[:], None,
                                op0=ALU.subtract)
        nc.gpsimd.dma_start(y.ap(), out_t[:])

    nc.compile()
    return nc


def _prep_inputs(x, W_ih0, W_ih_rest, W_hh, b_ih, b_hh, fc_w, fc_b, t_steps,
                 np_mm=None):
    """Host-side reshape/transpose into the layouts the kernel expects."""
    import ml_dtypes
    if np_mm is None:
        np_mm = ml_dtypes.bfloat16
    f = np.float32
    b_ih = np.asarray(b_ih, f)
    b_hh = np.asarray(b_hh, f)

    def brz(grp):
        rows = []
        for l in grp:
            for gate in range(2):
                rows.append(b_ih[l, gate * H:(gate + 1) * H]
                            + b_hh[l, gate * H:(gate + 1) * H])
        return np.ascontiguousarray(np.stack(rows).astype(np_mm))

    # rz ring sub-slots are gate-major: [r0 r1 r2 | z0 z1 z2]
    erz = np.zeros((6, 2 * 3 * 2 * PB), f)
    for k in range(6):
        j, gate = k // 2, k % 2
        for s in range(2):
            base = s * (3 * 2 * PB) + (gate * 3 + j) * PB
            erz[k, base:base + PB] = 1.0
    e3 = np.zeros((3, 2 * 3 * PB), f)
    for j in range(3):
        for s in range(2):
            base = s * (3 * PB) + j * PB
            e3[j, base:base + PB] = 1.0

    def bn(arr, grp):
        return np.ascontiguousarray(
            np.stack([arr[l, 2 * H:3 * H] for l in grp]).astype(np_mm))

    shared = {
        "wih0": np.ascontiguousarray(np.asarray(W_ih0, f).T.astype(np_mm)),
        "wih": np.ascontiguousarray(
            np.concatenate([np.asarray(W_ih_rest[l], f).T for l in range(L - 1)],
                           axis=1).astype(np_mm)),
        "whh": np.ascontiguousarray(
            np.concatenate([np.asarray(W_hh[l], f).T for l in range(L)],
                           axis=1).astype(np_mm)),
        "brz_a": brz(GRPS[0]),
        "brz_b": brz(GRPS[1]),
        "erz": np.ascontiguousarray(erz.astype(np_mm)),
        "bihn_a": bn(b_ih, GRPS[0]),
        "bihn_b": bn(b_ih, GRPS[1]),
        "e3": np.ascontiguousarray(e3.astype(np_mm)),
        "bhhn_a": bn(b_hh, GRPS[0]),
        "bhhn_b": bn(b_hh, GRPS[1]),
        "fcw": np.ascontiguousarray(np.asarray(fc_w, f).T.astype(np_mm)),
        "fcb": np.ascontiguousarray(np.asarray(fc_b, f).reshape(1, O).astype(np_mm)),
    }
    x = np.asarray(x, f)[:, :t_steps, :]
    in_maps = []
    for c in range(NCORES):
        xc = x[c * PB:(c + 1) * PB]                      # [PB, t, I]
        xT_c = np.ascontiguousarray(
            xc.transpose(2, 1, 0).reshape(I_DIM, t_steps * PB).astype(np_mm))
        in_maps.append({"xT": xT_c, **shared})
    return in_maps


def _run(nc, in_maps, trace=False):
    from concourse.bass_utils import run_bass_kernel_spmd
    return run_bass_kernel_spmd(nc, in_maps, core_ids=list(range(NCORES)),
                                trace=trace)


def kernel(x, W_ih0, W_ih_rest, W_hh, b_ih, b_hh, fc_w, fc_b):
    key = ("bf16", T)
    if key not in _CACHE:
        _CACHE[key] = _build(T, "bfloat16")
    nc = _CACHE[key]
    in_maps = _prep_inputs(x, W_ih0, W_ih_rest, W_hh, b_ih, b_hh, fc_w, fc_b, T)
    res = _run(nc, in_maps)
    return np.concatenate([res.results[c]["y"] for c in range(NCORES)], axis=0)


# revision 28
# speedup vs baseline: 1.1844x; 1.1844x over previous
"""Trainium2 Bass kernel for a 6-layer GRU network (B=256, T=512, I=28, H=128, O=10).

Strategy: data-parallel across 8 NeuronCores (batch 256 -> 32 per core),
with a 6-layer WAVEFRONT schedule inside each core: at wavefront step w,
layer l processes timestep t = w - 8*l.  The six layers are split into two
independent groups of three (layers 0-2 / 3-5) whose dependency chains
interleave on the engines, and all gate elementwise work is batched across
each group's three layers into [128, 96]-wide ops (vs [128, 32] per-layer).

Per group-step:
  - A PSUM "rz" ring (depth 4, refilled two steps ahead, after both
    groups' per-step ops so refills never head-of-line-block the PE
    queue) accumulates bias (K=6 selector matmul, start=True) + input
    projection (2-step chunks, strided dest, start=False) + recurrent
    W_hh matmuls (start=False), so ONE sigmoid op reads a contiguous
    [128,192] PSUM tile and emits r|z as bf16 to SBUF.
  - n-gate: gxn PSUM ring (bias + input proj), ghn PSUM ping-pong
    (bias + recurrent mm); hn2 = ghn * r and nin = hn2 + gxn on DVE
    (GpSimd cannot read PSUM); tanh on ScalarE.
  - h-update split so only two ops sit on the serial chain:
    u = z*h_prev and z' = 1-z run on GpSimd in parallel with the tanh,
    then h = u + z'*n takes two DVE ops.
  - h state lives in per-layer SBUF rings [128, L, 16, 32] indexed by
    wavefront slot (w % 16), so the batched 3-layer h-update writes one
    strided AP; each layer's t=0 step uses a zeroed slot / zeros rhs.
Final FC + log_softmax identical to the data-parallel baseline.
Measured: 1.78 ms HW exec (vs 6.71 ms for the per-layer sequential
baseline), rel err 4.4e-4.
"""

import numpy as np

H = 128
I_DIM = 28
L = 6
O = 10
B = 256
T = 512
NCORES = 8
PB = B // NCORES   # 32 batch rows per core
D_OFF = 4          # wavefront offset between consecutive layers
RING = 16          # h-state ring depth (slots of PB cols per layer)
GRPS = ([0, 1, 2], [3, 4, 5])

_CACHE = {}


def _build(t_steps, dt_mm_name="bfloat16"):
    from contextlib import ExitStack

    import concourse.bass as bass  # noqa: F401
    import concourse.tile as tile
    from concourse import bacc, mybir

    f32 = mybir.dt.float32
    bf16 = mybir.dt.bfloat16
    dt_mm = getattr(mybir.dt, dt_mm_name)
    AF = mybir.ActivationFunctionType
    ALU = mybir.AluOpType

    assert t_steps % 2 == 0
    w_end = t_steps + (L - 1) * D_OFF  # wavefront length

    nc = bacc.Bacc("TRN2", target_bir_lowering=False, debug=False)

    xT = nc.dram_tensor("xT", [I_DIM, PB * t_steps], dt_mm, kind="ExternalInput")
    wih0 = nc.dram_tensor("wih0", [I_DIM, 3 * H], dt_mm, kind="ExternalInput")
    wih = nc.dram_tensor("wih", [H, (L - 1) * 3 * H], dt_mm, kind="ExternalInput")
    whh = nc.dram_tensor("whh", [H, L * 3 * H], dt_mm, kind="ExternalInput")
    # rz bias rows per group: [6, H] (row k = layer grp[k//2], gate k%2 (r/z))
    brz_a = nc.dram_tensor("brz_a", [6, H], dt_mm, kind="ExternalInput")
    brz_b = nc.dram_tensor("brz_b", [6, H], dt_mm, kind="ExternalInput")
    erz = nc.dram_tensor("erz", [6, 2 * 3 * 2 * PB], dt_mm, kind="ExternalInput")
    bihn_a = nc.dram_tensor("bihn_a", [3, H], dt_mm, kind="ExternalInput")
    bihn_b = nc.dram_tensor("bihn_b", [3, H], dt_mm, kind="ExternalInput")
    e3 = nc.dram_tensor("e3", [3, 2 * 3 * PB], dt_mm, kind="ExternalInput")
    bhhn_a = nc.dram_tensor("bhhn_a", [3, H], dt_mm, kind="ExternalInput")
    bhhn_b = nc.dram_tensor("bhhn_b", [3, H], dt_mm, kind="ExternalInput")
    fcw = nc.dram_tensor("fcw", [H, O], dt_mm, kind="ExternalInput")
    fcb = nc.dram_tensor("fcb", [1, O], dt_mm, kind="ExternalInput")
    y = nc.dram_tensor("y", [PB, O], f32, kind="ExternalOutput")

    with tile.TileContext(nc) as tc, ExitStack() as ctx:
        consts = ctx.enter_context(tc.tile_pool(name="consts", bufs=1))
        # One persistent PSUM pool per group: rz ring (2 banks) + gxn ring
        # (1 bank) + ghn ping-pong (0.5 bank) = 4 banks; x2 groups = 8 banks.
        # Ring slots are padded so no matmul dest window crosses a bank.
        ps_pool = [
            ctx.enter_context(tc.tile_pool(name=f"ps_pool{g}", bufs=1, space="PSUM"))
            for g in range(2)
        ]
        rz_t = []
        gxn_t = []
        ghn_t = []
        for g in range(2):
            rz = ps_pool[g].tile([H, 4, 8, PB], f32, tag=f"rz{g}", name=f"rz{g}")
            gxn = ps_pool[g].tile([H, 4, 4, PB], f32, tag=f"gxn{g}", name=f"gxn{g}")
            ghn = ps_pool[g].tile([H, 2, 4, PB], f32, tag=f"ghn{g}", name=f"ghn{g}")
            rz_t.append(rz)
            gxn_t.append(gxn)
            ghn_t.append(ghn)
        rzsb_pool = ctx.enter_context(tc.tile_pool(name="rzsb", bufs=3))
        ew_pool = ctx.enter_context(tc.tile_pool(name="ew", bufs=3))
        scratch = ctx.enter_context(tc.tile_pool(name="scratch", bufs=3))

        # ---- load constants ----
        xT_sb = consts.tile([I_DIM, PB * t_steps], dt_mm, tag="xT_sb")
        nc.gpsimd.dma_start(xT_sb[:], xT.ap())
        wih0_sb = consts.tile([I_DIM, 3 * H], dt_mm, tag="wih0_sb")
        nc.gpsimd.dma_start(wih0_sb[:], wih0.ap())
        wih_sb = consts.tile([H, (L - 1) * 3 * H], dt_mm, tag="wih_sb")
        nc.gpsimd.dma_start(wih_sb[:], wih.ap())
        whh_sb = consts.tile([H, L * 3 * H], dt_mm, tag="whh_sb")
        nc.gpsimd.dma_start(whh_sb[:], whh.ap())
        brz_sb = [consts.tile([6, H], dt_mm, tag=f"brz{g}_sb", name=f"brz{g}_sb") for g in range(2)]
        nc.gpsimd.dma_start(brz_sb[0][:], brz_a.ap())
        nc.gpsimd.dma_start(brz_sb[1][:], brz_b.ap())
        erz_sb = consts.tile([6, 2 * 3 * 2 * PB], dt_mm, tag="erz_sb")
        nc.gpsimd.dma_start(erz_sb[:], erz.ap())
        bihn_sb = [consts.tile([3, H], dt_mm, tag=f"bihn{g}_sb", name=f"bihn{g}_sb") for g in range(2)]
        nc.gpsimd.dma_start(bihn_sb[0][:], bihn_a.ap())
        nc.gpsimd.dma_start(bihn_sb[1][:], bihn_b.ap())
        e3_sb = consts.tile([3, 2 * 3 * PB], dt_mm, tag="e3_sb")
        nc.gpsimd.dma_start(e3_sb[:], e3.ap())
        bhhn_sb = [consts.tile([3, H], dt_mm, tag=f"bhhn{g}_sb", name=f"bhhn{g}_sb") for g in range(2)]
        nc.gpsimd.dma_start(bhhn_sb[0][:], bhhn_a.ap())
        nc.gpsimd.dma_start(bhhn_sb[1][:], bhhn_b.ap())
        fcw_sb = consts.tile([H, O], dt_mm, tag="fcw_sb")
        nc.gpsimd.dma_start(fcw_sb[:], fcw.ap())
        fcb_sb = consts.tile([1, O], dt_mm, tag="fcb_sb")
        nc.gpsimd.dma_start(fcb_sb[:], fcb.ap())

        zeros_sb = consts.tile([H, PB], dt_mm, tag="zeros_sb")
        nc.vector.memset(zeros_sb[:], 0.0)
        ones_sb = consts.tile([1, PB], dt_mm, tag="ones_sb")
        nc.vector.memset(ones_sb[:], 1.0)

        # h-state rings: [128, layer, slot, batch]; slot = w % RING
        h_ring = consts.tile([H, L, RING, PB], dt_mm, tag="h_ring")
        nc.vector.memset(h_ring[:], 0.0)

        def whh_g(layer, g):
            return whh_sb[:, (layer * 3 + g) * H:(layer * 3 + g + 1) * H]

        def wih_g(layer, g):
            if layer == 0:
                return wih0_sb[:, g * H:(g + 1) * H]
            base = ((layer - 1) * 3 + g) * H
            return wih_sb[:, base:base + H]

        def emit_refill(g, grp, wf):
            """Fill rz + gxn ring slots for steps wf, wf+1 (bias + inproj)."""
            rf = wf % 4
            nc.tensor.matmul(rz_t[g][:, rf:rf + 2, 0:6, :],
                             brz_sb[g][:], erz_sb[:],
                             start=True, stop=False)
            nc.tensor.matmul(gxn_t[g][:, rf:rf + 2, 0:3, :],
                             bihn_sb[g][:], e3_sb[:],
                             start=True, stop=False)
            for j, l in enumerate(grp):
                if l == 0:
                    tc0 = min(wf, t_steps - 2)
                    rhs = xT_sb[:, tc0 * PB:(tc0 + 2) * PB]
                else:
                    ps = (wf - D_OFF) % RING  # even, no wrap
                    rhs = h_ring[:, l - 1, ps:ps + 2, :]
                for gate in range(2):
                    nc.tensor.matmul(rz_t[g][:, rf:rf + 2, j * 2 + gate, :],
                                     wih_g(l, gate), rhs,
                                     start=False, stop=False,
                                     skip_group_check=True)
                nc.tensor.matmul(gxn_t[g][:, rf:rf + 2, j, :],
                                 wih_g(l, 2), rhs,
                                 start=False, stop=True,
                                 skip_group_check=True)

        for g, grp in enumerate(GRPS):
            emit_refill(g, grp, 0)
            emit_refill(g, grp, 2)

        for w in range(w_end):
            s2 = w % 2
            r4 = w % 4
            slot = w % RING
            # Phase-interleaved emission: per engine queue, both groups'
            # phase-k ops precede any group's phase-k+1 op, so group B's
            # sigmoid is never stuck behind group A's (dependency-waiting)
            # tanh in the ACT queue, etc.
            prev_slot = (w - 1) % RING
            st = [{}, {}]
            for g, grp in enumerate(GRPS):
                # ---- phase 1: recurrent matmuls (rz first: the sigmoid
                # waits only on them) ----
                ghn = ghn_t[g][:, s2]  # [H, 4, PB] ping-pong slot
                nc.tensor.matmul(ghn[:, 0:3, :], bhhn_sb[g][:],
                                 e3_sb[:, 0:3 * PB], start=True, stop=False)
                hprevs = []
                for j, l in enumerate(grp):
                    t_l = w - D_OFF * l
                    if t_l == 0:
                        hprev = zeros_sb[:]
                    else:
                        hprev = h_ring[:, l, prev_slot, :]
                    hprevs.append(hprev)
                    nc.tensor.matmul(rz_t[g][:, r4, j * 2, :], whh_g(l, 0), hprev,
                                     start=False, stop=True,
                                     skip_group_check=True)
                    nc.tensor.matmul(rz_t[g][:, r4, j * 2 + 1, :], whh_g(l, 1), hprev,
                                     start=False, stop=True,
                                     skip_group_check=True)
                for j, l in enumerate(grp):
                    nc.tensor.matmul(ghn[:, j, :], whh_g(l, 2), hprevs[j],
                                     start=False, stop=True,
                                     skip_group_check=True)
                st[g]["ghn"] = ghn
            for g, grp in enumerate(GRPS):
                # ---- phase 2: sigmoid ----
                rz_sb = rzsb_pool.tile([H, 3, 2, PB], dt_mm, tag=f"rzsb{g}",
                                       name=f"rzsb{g}")
                nc.scalar.activation(rz_sb[:], rz_t[g][:, r4, 0:6, :], AF.Sigmoid)
                st[g]["r"] = rz_sb[:, :, 0, :]
                st[g]["z"] = rz_sb[:, :, 1, :]
            for g, grp in enumerate(GRPS):
                # ---- phase 3: n-gate input on DVE ----
                hn2_sb = ew_pool.tile([H, 3, PB], dt_mm, tag=f"hn2{g}",
                                      name=f"hn2{g}")
                nc.vector.tensor_tensor(hn2_sb[:], st[g]["ghn"][:, 0:3, :],
                                        st[g]["r"], op=ALU.mult)
                nin_sb = ew_pool.tile([H, 3, PB], dt_mm, tag=f"nin{g}",
                                      name=f"nin{g}")
                nc.vector.tensor_tensor(nin_sb[:], hn2_sb[:],
                                        gxn_t[g][:, r4, 0:3, :], op=ALU.add)
                st[g]["nin"] = nin_sb
            for g, grp in enumerate(GRPS):
                # ---- phase 4 (off-chain, Pool): u = z*h_prev, z' = 1-z ----
                hprev3 = h_ring[:, grp[0]:grp[0] + 3, prev_slot, :]
                u_sb = ew_pool.tile([H, 3, PB], dt_mm, tag=f"u{g}", name=f"u{g}")
                nc.gpsimd.tensor_tensor(u_sb[:], st[g]["z"], hprev3, op=ALU.mult)
                zc_sb = ew_pool.tile([H, 3, PB], dt_mm, tag=f"zc{g}", name=f"zc{g}")
                nc.gpsimd.tensor_scalar(zc_sb[:], st[g]["z"], -1.0, 1.0,
                                        op0=ALU.mult, op1=ALU.add)
                st[g]["u"], st[g]["zc"] = u_sb, zc_sb
            for g, grp in enumerate(GRPS):
                # ---- phase 5: tanh ----
                n_sb = ew_pool.tile([H, 3, PB], dt_mm, tag=f"n{g}", name=f"n{g}")
                nc.scalar.activation(n_sb[:], st[g]["nin"][:], AF.Tanh)
                st[g]["n"] = n_sb
            for g, grp in enumerate(GRPS):
                # ---- phase 6: on-chain tail h = u + (1-z)*n ----
                v_sb = ew_pool.tile([H, 3, PB], dt_mm, tag=f"v{g}", name=f"v{g}")
                nc.vector.tensor_tensor(v_sb[:], st[g]["zc"][:], st[g]["n"][:],
                                        op=ALU.mult)
                nc.vector.tensor_tensor(h_ring[:, grp[0]:grp[0] + 3, slot, :],
                                        st[g]["u"][:], v_sb[:], op=ALU.add)
                # zero the slot a layer will read as h(-1) at its t=0
                for l in grp:
                    if l > 0 and w == D_OFF * l - 1:
                        nc.vector.memset(h_ring[:, l, slot, :], 0.0)

            # ---- refill ring slots 2 steps AHEAD (the ring is depth 4:
            # slots of steps w-2/w-1, already emitted), AFTER both groups'
            # per-step ops so refills never head-of-line-block a group's
            # recurrent matmuls in the PE queue ----
            if s2 == 0 and w >= 2:
                for g, grp in enumerate(GRPS):
                    emit_refill(g, grp, w + 2)

        # ---- FC + log_softmax on h(L-1, T-1) ----
        h_last = h_ring[:, L - 1, (w_end - 1) % RING, :]
        # reuse an untouched pad column of group 1's ghn bank for the logits
        logits_ps = ghn_t[1][0:PB, 1, 3, 0:O]
        nc.tensor.matmul(logits_ps, h_last, fcw_sb[:], start=True, stop=False)
        nc.tensor.matmul(logits_ps, ones_sb[:], fcb_sb[:],
                         start=False, stop=True, skip_group_check=True)
        mx_t = scratch.tile([PB, 1], f32, tag="mx")
        nc.vector.reduce_max(mx_t[:], logits_ps, axis=mybir.AxisListType.X)
        xm_t = scratch.tile([PB, O], f32, tag="xm")
        nc.vector.tensor_scalar(xm_t[:], logits_ps, mx_t[:], None,
                                op0=ALU.subtract)
        ex_t = scratch.tile([PB, O], f32, tag="ex")
        sum_t = scratch.tile([PB, 1], f32, tag="sum")
        nc.scalar.activation(ex_t[:], xm_t[:], AF.Exp, accum_out=sum_t[:])
        ls_t = scratch.tile([PB, 1], f32, tag="ls")
        nc.scalar.activation(ls_t[:], sum_t[:], AF.Ln)
        out_t = scratch.tile([PB, O], f32, tag="out")
        nc.vector.tensor_scalar(out_t[:], xm_t[:], ls_t[:], None,
                                op0=ALU.subtract)
        nc.gpsimd.dma_start(y.ap(), out_t[:])

    nc.compile()
    return nc


def _prep_inputs(x, W_ih0, W_ih_rest, W_hh, b_ih, b_hh, fc_w, fc_b, t_steps,
                 np_mm=None):
    """Host-side reshape/transpose into the layouts the kernel expects."""
    import ml_dtypes
    if np_mm is None:
        np_mm = ml_dtypes.bfloat16
    f = np.float32
    b_ih = np.asarray(b_ih, f)
    b_hh = np.asarray(b_hh, f)

    def brz(grp):
        rows = []
        for l in grp:
            for gate in range(2):
                rows.append(b_ih[l, gate * H:(gate + 1) * H]
                            + b_hh[l, gate * H:(gate + 1) * H])
        return np.ascontiguousarray(np.stack(rows).astype(np_mm))

    erz = np.zeros((6, 2 * 3 * 2 * PB), f)
    for k in range(6):
        for s in range(2):
            base = s * (3 * 2 * PB) + k * PB
            erz[k, base:base + PB] = 1.0
    e3 = np.zeros((3, 2 * 3 * PB), f)
    for j in range(3):
        for s in range(2):
            base = s * (3 * PB) + j * PB
            e3[j, base:base + PB] = 1.0

    def bn(arr, grp):
        return np.ascontiguousarray(
            np.stack([arr[l, 2 * H:3 * H] for l in grp]).astype(np_mm))

    shared = {
        "wih0": np.ascontiguousarray(np.asarray(W_ih0, f).T.astype(np_mm)),
        "wih": np.ascontiguousarray(
            np.concatenate([np.asarray(W_ih_rest[l], f).T for l in range(L - 1)],
                           axis=1).astype(np_mm)),
        "whh": np.ascontiguousarray(
            np.concatenate([np.asarray(W_hh[l], f).T for l in range(L)],
                           axis=1).astype(np_mm)),
        "brz_a": brz(GRPS[0]),
        "brz_b": brz(GRPS[1]),
        "erz": np.ascontiguousarray(erz.astype(np_mm)),
        "bihn_a": bn(b_ih, GRPS[0]),
        "bihn_b": bn(b_ih, GRPS[1]),
        "e3": np.ascontiguousarray(e3.astype(np_mm)),
        "bhhn_a": bn(b_hh, GRPS[0]),
        "bhhn_b": bn(b_hh, GRPS[1]),
        "fcw": np.ascontiguousarray(np.asarray(fc_w, f).T.astype(np_mm)),
        "fcb": np.ascontiguousarray(np.asarray(fc_b, f).reshape(1, O).astype(np_mm)),
    }
    x = np.asarray(x, f)[:, :t_steps, :]
    in_maps = []
    for c in range(NCORES):
        xc = x[c * PB:(c + 1) * PB]                      # [PB, t, I]
        xT_c = np.ascontiguousarray(
            xc.transpose(2, 1, 0).reshape(I_DIM, t_steps * PB).astype(np_mm))
        in_maps.append({"xT": xT_c, **shared})
    return in_maps


def _run(nc, in_maps, trace=False):
    from concourse.bass_utils import run_bass_kernel_spmd
    return run_bass_kernel_spmd(nc, in_maps, core_ids=list(range(NCORES)),
                                trace=trace)


def kernel(x, W_ih0, W_ih_rest, W_hh, b_ih, b_hh, fc_w, fc_b):
    key = ("bf16", T)
    if key not in _CACHE:
        _CACHE[key] = _build(T, "bfloat16")
    nc = _CACHE[key]
    in_maps = _prep_inputs(x, W_ih0, W_ih_rest, W_hh, b_ih, b_hh, fc_w, fc_b, T)
    res = _run(nc, in_maps)
    return np.concatenate([res.results[c]["y"] for c in range(NCORES)], axis=0)
